# revision 26
# baseline (speedup 1.0000x reference)
"""Bass/Trainium2 kernel for nn_DifferentSoftQNetwork.

Math: the reference is three chained per-sample-expert matmuls with NO
nonlinearity:
    out[b] = state[b] @ W1[o_b] @ W2[o_b] @ W3[o_b],   o_b = option[b]
Because it is linear, collapse the weight chain per expert:
    v[e] = W1[e] @ W2[e] @ W3[e]  in R^128
    out[b] = dot(state[b], v[o_b])
This turns a 672-MFLOP batched matmul into a memory-bound stream of the
weights (~21 MB).

Sharding: experts are sharded across the 8 cores (2 experts per core);
state/option are replicated.  Each core computes scores s[e, b] for its two
experts and masks them by (option == e); the host sums the 8 partial
[2, 1024] outputs (each b matches exactly one (core, expert) pair).

Input marshalling on the host passes state and W1 pre-transposed (stateT
[I, B], W1T [E, H, I]) so every device-side matmul consumes its natural
layout directly (PE matmul computes lhsT.T @ rhs and needs the contraction
dim on partitions for both operands).

Per-core device program (experts e0, e1):
  CT[e]     = W1T[e].T @ W2[e] = W1[e] @ W2[e]    ([128i, 512k] in PSUM)
  w3b[e]    = partition_broadcast(W3[e])          (GpSimd)
  V[:, e]   = reduce_k(CT[e] * w3b[e])            (DVE mul + reduce)
  sT        = V^T @ ST                            ([2, 1024] scores)
  out[e, b] = sT[e, b] * (option[b] == e)         (DVE is_equal + mul)

DMA issue is split across the two HWDGE queues (sync + scalar) with few,
large transfers; the host-side gather sums the partial outputs.
"""

import numpy as np

B, I, H, O = 1024, 128, 512, 16
NCORES = 8
EPC = O // NCORES  # experts per core = 2

_CACHE = {}

# Internal knobs for the local test harness (the grading harness never
# touches these): when _TRACE is set, the next kernel() call runs with
# NTFF profiling and stores the BassKernelResults in _LAST_RESULTS.
_TRACE = False
_LAST_RESULTS = None

# dummy PE matmuls at kernel start to lift the HAM clock gate
N_WARMUP = 6
N_WARMUP2 = 0


def _build_nc():
    import concourse.bacc as bacc
    import concourse.bass as bass
    import concourse.mybir as mybir
    import concourse.tile as tile

    fp32 = mybir.dt.float32
    fp32r = mybir.dt.float32r
    P = 128

    nc = bacc.Bacc("TRN2", target_bir_lowering=False, debug=False)

    statet_d = nc.dram_tensor("statet", [I, B], fp32r, kind="ExternalInput")
    w1t_d = nc.dram_tensor("w1t", [EPC, H, I], fp32r, kind="ExternalInput")
    w2_d = nc.dram_tensor("w2", [EPC, H, H], fp32r, kind="ExternalInput")
    w3_d = nc.dram_tensor("w3", [EPC, H], fp32r, kind="ExternalInput")
    optf_d = nc.dram_tensor("optf", [1, B], fp32, kind="ExternalInput")
    ones_d = nc.dram_tensor("ones", [1, P], fp32r, kind="ExternalInput")
    ce_d = nc.dram_tensor("ce01", [EPC, 1], fp32, kind="ExternalInput")
    out_d = nc.dram_tensor("outp", [EPC, B], fp32, kind="ExternalOutput")

    NH = H // P  # 4 chunks of the hidden dim
    NS = 512  # max moving free dim per matmul

    with tile.TileContext(nc) as tc:
        with (
            tc.tile_pool(name="sb", bufs=1) as sb,
            tc.tile_pool(name="sb2", bufs=2) as sb2,
            tc.tile_pool(name="psc", bufs=2, space=bass.MemorySpace.PSUM) as psc,
            tc.tile_pool(name="pss", bufs=2, space=bass.MemorySpace.PSUM) as pss,
        ):
            # ---- DMA loads.  The two HWDGE queues (sync = expert 0,
            # scalar = expert 1) stream, in first-use order: w3 row + ones,
            # W1T, W2 chunks (optf slipped in mid-queue).  state^T goes on
            # the GpSimd SWDGE queue, issued first — it is only needed by
            # the score matmuls near the end, and keeping it off the HWDGE
            # queues shortens the critical W2 delivery.
            engs = [nc.sync, nc.scalar]
            w1t_view = w1t_d.rearrange("e (c p) i -> e p c i", p=P)
            w2_view = w2_d.rearrange("e (h p) k -> e p h k", p=P)
            w3rs, w1ts, w2s = [], [], []
            ST = sb.tile([I, B], fp32r, tag="ST", name="ST")
            optf = sb.tile([EPC, B], fp32, tag="optf", name="optf")
            ce01 = sb.tile([EPC, 1], fp32, tag="ce01", name="ce01")
            ones = sb.tile([1, P], fp32r, tag="ones", name="ones")
            half = B // EPC
            # SWDGE (gpsimd): state^T halves + every small tensor, issued
            # up front; none of these are needed before ~mid-kernel.
            for e in range(EPC):
                nc.gpsimd.dma_start(
                    ST[:, e * half : (e + 1) * half],
                    statet_d[:, e * half : (e + 1) * half],
                )
            for e in range(EPC):
                t = sb.tile([1, H], fp32r, tag=f"w3r_{e}", name=f"w3r_{e}")
                nc.gpsimd.dma_start(t[:], w3_d[e : e + 1, :])
                w3rs.append(t)
            nc.gpsimd.dma_start(ones[:], ones_d[:])
            nc.gpsimd.dma_start(optf[:], optf_d[0:1, :].to_broadcast([EPC, B]))
            nc.gpsimd.dma_start(ce01[:], ce_d[:])
            # HWDGE queues: pure W1T + W2 per expert.
            for e in range(EPC):
                eng = engs[e]
                t = sb.tile([P, NH * I], fp32r, tag=f"w1t_{e}", name=f"w1t_{e}")
                eng.dma_start(t[:], w1t_view[e])
                w1ts.append(t)
                w2t = sb.tile([P, NH * H], fp32r, tag=f"w2_{e}", name=f"w2_{e}")
                for h in range(NH):
                    eng.dma_start(
                        w2t[:, h * H : (h + 1) * H], w2_view[e][:, h, :]
                    )
                w2s.append(w2t)

            # ---- PE warm-up: dummy matmuls so the HAM clock gate reaches
            # 2.4 GHz before the real contraction starts.
            wz = sb.tile([P, 256], fp32, tag="wz", name="wz")
            nc.vector.memset(wz[:], 0.0)
            wp = psc.tile([P, 256], fp32, tag="wp", name="wp", bufs=1)
            for _ in range(N_WARMUP):
                nc.tensor.matmul(wp[:], wz[:, :P], wz[:], start=True, stop=True)

            # ---- W3 rows broadcast across partitions via PE (ones outer
            # product, fp32r single pass), then copied to SBUF.
            w3bs = []
            for e in range(EPC):
                w3p = psc.tile([P, H], fp32, tag="w3p", name="w3p")
                nc.tensor.matmul(w3p[:], ones[:], w3rs[e][:], start=True, stop=True)
                t = sb.tile([P, H], fp32, tag=f"w3b_{e}", name=f"w3b_{e}")
                nc.vector.tensor_copy(t[:], w3p[:])
                w3bs.append(t)

            # ---- selection masks (early; only needs optf).  The host
            # passes optf pre-shifted by the core's expert base, so row e
            # compares against the constant e (same program on all cores).
            eq = sb.tile([EPC, B], fp32, tag="eq", name="eq")
            nc.vector.tensor_scalar(
                eq[:], optf[:], ce01[:], None, op0=mybir.AluOpType.is_equal
            )

            # ---- per expert: CT[e] = W1[e] @ W2[e]; V[:,e] = CT[e] @ W3[e]
            V = sb.tile([P, EPC], fp32r, tag="V", name="V")
            for e in range(EPC):
                ct = psc.tile([P, H], fp32, tag="ct", name="ct")
                for h in range(NH):
                    nc.tensor.matmul(
                        ct[:],
                        w1ts[e][:, h * I : (h + 1) * I],
                        w2s[e][:, h * H : (h + 1) * H],
                        start=(h == 0),
                        stop=(h == NH - 1),
                    )
                junk = sb2.tile([P, H], fp32, tag="junk", name="junk")
                with nc.allow_low_precision(reason="fp32r V for fast PE"):
                    nc.vector.scalar_tensor_tensor(
                        junk[:],
                        ct[:],
                        1.0,
                        w3bs[e][:],
                        op0=mybir.AluOpType.mult,
                        op1=mybir.AluOpType.mult,
                        accum_out=V[:, e : e + 1],
                    )

            # ---- scores for both experts at once, then masked output
            outp = sb.tile([EPC, B], fp32, tag="outp", name="outp")
            for half in range(B // NS):
                stp = pss.tile([EPC, NS], fp32, tag="stp", name="stp")
                nc.tensor.matmul(
                    stp[:],
                    V[:],
                    ST[:, half * NS : (half + 1) * NS],
                    start=True,
                    stop=True,
                )
                nc.vector.tensor_mul(
                    outp[:, half * NS : (half + 1) * NS],
                    stp[:],
                    eq[:, half * NS : (half + 1) * NS],
                )
            nc.sync.dma_start(out_d[:], outp[:])

    nc.compile()
    return nc


def _get_nc():
    if "nc" not in _CACHE:
        _CACHE["nc"] = _build_nc()
    return _CACHE["nc"]


def kernel(state, action, W1, W2, W3, option):
    global _LAST_RESULTS
    from concourse import bass_utils

    nc = _get_nc()

    state = np.asarray(state, dtype=np.float32)
    statet = np.ascontiguousarray(state.T)
    W1 = np.asarray(W1, dtype=np.float32)
    w1t = np.ascontiguousarray(np.transpose(W1, (0, 2, 1)))  # [O, H, I]
    W2 = np.asarray(W2, dtype=np.float32)
    W3 = np.asarray(W3, dtype=np.float32)
    opt = np.asarray(option).astype(np.float32).reshape(1, B)

    in_maps = []
    for c in range(NCORES):
        e0 = EPC * c
        in_maps.append(
            {
                "statet": statet,
                "w1t": np.ascontiguousarray(w1t[e0 : e0 + EPC]),
                "w2": np.ascontiguousarray(W2[e0 : e0 + EPC]),
                "w3": np.ascontiguousarray(W3[e0 : e0 + EPC, :, 0]),
                "optf": opt - np.float32(e0),
                "ce01": np.arange(EPC, dtype=np.float32).reshape(EPC, 1),
                "ones": np.ones((1, 128), dtype=np.float32),
            }
        )

    res = bass_utils.run_bass_kernel_spmd(
        nc, in_maps, core_ids=list(range(NCORES)), trace=_TRACE
    )
    _LAST_RESULTS = res

    out = np.zeros((B,), np.float32)
    for c in range(NCORES):
        out += res.results[c]["outp"].sum(axis=0)
    return out.reshape(B, 1)


# revision 27
# speedup vs baseline: 1.1205x; 1.1205x over previous
"""Bass/Trainium2 kernel for nn_DifferentSoftQNetwork.

Math: the reference is three chained per-sample-expert matmuls with NO
nonlinearity:
    out[b] = state[b] @ W1[o_b] @ W2[o_b] @ W3[o_b],   o_b = option[b]
Because it is linear, collapse the weight chain per expert:
    v[e] = W1[e] @ W2[e] @ W3[e]  in R^128
    out[b] = dot(state[b], v[o_b])
This turns a 672-MFLOP batched matmul into a memory-bound stream of the
weights (~21 MB).

Sharding: experts are sharded across the 8 cores (2 experts per core);
state/option are replicated.  Each core computes scores s[e, b] for its two
experts and masks them by (option == e); the host sums the 8 partial
[2, 1024] outputs (each b matches exactly one (core, expert) pair).

Host-side input marshalling: state and W1 are passed pre-transposed
(stateT [I, B], W1T [E, H, I]) so every device matmul consumes its natural
layout (PE computes lhsT.T @ rhs with the contraction dim on partitions for
both operands); option is passed as f32 pre-shifted by the core's expert
base so the device compares against constants 0/1 (same SPMD program on
every core).

The big matmul operands are typed float32r (full 4-byte data; the PE's
fast fp32 path) — 1 cycle/row instead of 4, costing ~1e-4 relative error,
far inside the accuracy gate.

Per-core device program (experts e0, e1):
  warm-up matmuls                              (lift the PE HAM clock gate)
  w3b[e] = ones^T @ W3[e]                      (PE broadcast, fp32r)
  CT[e]  = W1T[e].T @ W2[e] = W1[e] @ W2[e]    ([128i, 512k] in PSUM)
  V[:,e] = reduce_k(CT[e] * w3b[e])            (DVE fused mul+reduce)
  sT     = V^T @ ST                            ([2, 1024] scores, fp32r)
  out[e] = sT[e] * (option == e)               (DVE is_equal mask + mul)

DMA: the two HWDGE queues (sync, scalar) each stream one expert's
W1T + W2 chunks (the critical path) with the tiny w3/ones rows in front
and state^T halves + optf at the tail; ~3 MB/core total, which saturates
the ~320 GB/s per-core DMA ceiling.
"""

import numpy as np

B, I, H, O = 1024, 128, 512, 16
NCORES = 8
EPC = O // NCORES  # experts per core = 2

_CACHE = {}

# Internal knobs for the local test harness (the grading harness never
# touches these): when _TRACE is set, the next kernel() call runs with
# NTFF profiling and stores the BassKernelResults in _LAST_RESULTS.
_TRACE = False
_LAST_RESULTS = None

# dummy PE matmuls at kernel start to lift the HAM clock gate
N_WARMUP = 6


def _build_nc():
    import concourse.bacc as bacc
    import concourse.bass as bass
    import concourse.mybir as mybir
    import concourse.tile as tile

    fp32 = mybir.dt.float32
    fp32r = mybir.dt.float32r
    P = 128

    nc = bacc.Bacc("TRN2", target_bir_lowering=False, debug=False)

    statet_d = nc.dram_tensor("statet", [I, B], fp32r, kind="ExternalInput")
    w1t_d = nc.dram_tensor("w1t", [EPC, H, I], fp32r, kind="ExternalInput")
    w2_d = nc.dram_tensor("w2", [EPC, H, H], fp32r, kind="ExternalInput")
    w3_d = nc.dram_tensor("w3", [EPC, H], fp32r, kind="ExternalInput")
    optf_d = nc.dram_tensor("optf", [1, B], fp32, kind="ExternalInput")
    ones_d = nc.dram_tensor("ones", [1, P], fp32r, kind="ExternalInput")
    ce_d = nc.dram_tensor("ce01", [EPC, 1], fp32, kind="ExternalInput")
    out_d = nc.dram_tensor("outp", [EPC, B], fp32, kind="ExternalOutput")

    NH = H // P  # 4 chunks of the hidden dim
    NS = 512  # max moving free dim per matmul

    with tile.TileContext(nc) as tc:
        with (
            tc.tile_pool(name="sb", bufs=1) as sb,
            tc.tile_pool(name="sb2", bufs=2) as sb2,
            tc.tile_pool(name="psc", bufs=2, space=bass.MemorySpace.PSUM) as psc,
            tc.tile_pool(name="pss", bufs=2, space=bass.MemorySpace.PSUM) as pss,
        ):
            engs = [nc.sync, nc.scalar]
            w1t_view = w1t_d.rearrange("e (c p) i -> e p c i", p=P)
            w2_view = w2_d.rearrange("e (h p) k -> e p h k", p=P)
            w3rs, w1ts, w2s = [], [], []
            ST = sb.tile([I, B], fp32r, tag="ST", name="ST")
            optf = sb.tile([EPC, B], fp32, tag="optf", name="optf")
            ce01 = sb.tile([EPC, 1], fp32, tag="ce01", name="ce01")
            ones = sb.tile([1, P], fp32r, tag="ones", name="ones")
            half = B // EPC
            for e in range(EPC):
                eng = engs[e]
                t = sb.tile([1, H], fp32r, tag=f"w3r_{e}", name=f"w3r_{e}")
                eng.dma_start(t[:], w3_d[e : e + 1, :])
                w3rs.append(t)
                if e == 0:
                    eng.dma_start(ones[:], ones_d[:])
                t = sb.tile([P, NH * I], fp32r, tag=f"w1t_{e}", name=f"w1t_{e}")
                eng.dma_start(t[:], w1t_view[e])
                w1ts.append(t)
                w2t = sb.tile([P, NH * H], fp32r, tag=f"w2_{e}", name=f"w2_{e}")
                for h in range(NH):
                    eng.dma_start(
                        w2t[:, h * H : (h + 1) * H], w2_view[e][:, h, :]
                    )
                w2s.append(w2t)
            nc.scalar.dma_start(optf[:], optf_d[0:1, :].to_broadcast([EPC, B]))
            nc.scalar.dma_start(ce01[:], ce_d[:])
            for e in range(EPC):
                engs[e].dma_start(
                    ST[:, e * half : (e + 1) * half],
                    statet_d[:, e * half : (e + 1) * half],
                )

            # ---- PE warm-up: dummy matmuls so the HAM clock gate reaches
            # 2.4 GHz before the real contraction starts.
            wz = sb.tile([P, 256], fp32, tag="wz", name="wz")
            nc.vector.memset(wz[:], 0.0)
            wp = psc.tile([P, 256], fp32, tag="wp", name="wp", bufs=1)
            for _ in range(N_WARMUP):
                nc.tensor.matmul(wp[:], wz[:, :P], wz[:], start=True, stop=True)

            # ---- W3 rows broadcast across partitions via PE (ones outer
            # product, fp32r single pass), then copied to SBUF.
            w3bs = []
            for e in range(EPC):
                w3p = psc.tile([P, H], fp32, tag="w3p", name="w3p")
                nc.tensor.matmul(w3p[:], ones[:], w3rs[e][:], start=True, stop=True)
                t = sb.tile([P, H], fp32, tag=f"w3b_{e}", name=f"w3b_{e}")
                nc.vector.tensor_copy(t[:], w3p[:])
                w3bs.append(t)

            # ---- selection masks (needs only optf + ce01)
            eq = sb.tile([EPC, B], fp32, tag="eq", name="eq")
            nc.vector.tensor_scalar(
                eq[:], optf[:], ce01[:], None, op0=mybir.AluOpType.is_equal
            )

            # ---- per expert: CT[e] = W1[e] @ W2[e] accumulated in PSUM,
            # then V[:,e] = reduce_k(CT[e] * w3b[e]) in one DVE op.
            V = sb.tile([P, EPC], fp32r, tag="V", name="V")
            for e in range(EPC):
                ct = psc.tile([P, H], fp32, tag="ct", name="ct")
                for h in range(NH):
                    nc.tensor.matmul(
                        ct[:],
                        w1ts[e][:, h * I : (h + 1) * I],
                        w2s[e][:, h * H : (h + 1) * H],
                        start=(h == 0),
                        stop=(h == NH - 1),
                    )
                junk = sb2.tile([P, H], fp32, tag="junk", name="junk")
                with nc.allow_low_precision(reason="fp32r V for fast PE"):
                    nc.vector.scalar_tensor_tensor(
                        junk[:],
                        ct[:],
                        1.0,
                        w3bs[e][:],
                        op0=mybir.AluOpType.mult,
                        op1=mybir.AluOpType.mult,
                        accum_out=V[:, e : e + 1],
                    )

            # ---- scores for both experts at once, then masked output;
            # each half ships as soon as its mask-mul finishes.
            for hf in range(B // NS):
                stp = pss.tile([EPC, NS], fp32, tag="stp", name="stp")
                nc.tensor.matmul(
                    stp[:],
                    V[:],
                    ST[:, hf * NS : (hf + 1) * NS],
                    start=True,
                    stop=True,
                )
                outp = sb2.tile([EPC, NS], fp32, tag="outp", name="outp")
                nc.vector.tensor_mul(outp[:], stp[:], eq[:, hf * NS : (hf + 1) * NS])
                nc.sync.dma_start(out_d[:, hf * NS : (hf + 1) * NS], outp[:])

    nc.compile()
    return nc


def _get_nc():
    if "nc" not in _CACHE:
        _CACHE["nc"] = _build_nc()
    return _CACHE["nc"]


def kernel(state, action, W1, W2, W3, option):
    global _LAST_RESULTS
    from concourse import bass_utils

    nc = _get_nc()

    state = np.asarray(state, dtype=np.float32)
    statet = np.ascontiguousarray(state.T)
    W1 = np.asarray(W1, dtype=np.float32)
    w1t = np.ascontiguousarray(np.transpose(W1, (0, 2, 1)))  # [O, H, I]
    W2 = np.asarray(W2, dtype=np.float32)
    W3 = np.asarray(W3, dtype=np.float32)
    opt = np.asarray(option).astype(np.float32).reshape(1, B)

    in_maps = []
    for c in range(NCORES):
        e0 = EPC * c
        in_maps.append(
            {
                "statet": statet,
                "w1t": np.ascontiguousarray(w1t[e0 : e0 + EPC]),
                "w2": np.ascontiguousarray(W2[e0 : e0 + EPC]),
                "w3": np.ascontiguousarray(W3[e0 : e0 + EPC, :, 0]),
                "optf": opt - np.float32(e0),
                "ones": np.ones((1, 128), dtype=np.float32),
                "ce01": np.arange(EPC, dtype=np.float32).reshape(EPC, 1),
            }
        )

    res = bass_utils.run_bass_kernel_spmd(
        nc, in_maps, core_ids=list(range(NCORES)), trace=_TRACE
    )
    _LAST_RESULTS = res

    out = np.zeros((B,), np.float32)
    for c in range(NCORES):
        out += res.results[c]["outp"].sum(axis=0)
    return out.reshape(B, 1)
